# revision 35
# baseline (speedup 1.0000x reference)
"""Trainium2 Bass kernel for nn_HFMelSpectrogram.

Pipeline (per core, 4 batches of the 32-batch waveform):
  1. STFT-as-GEMM in fp8e4m3 with DoubleRow perf mode: the PE array is
     virtualized to 128x256, so the 1024-deep contraction takes 4 matmuls
     (each pairs two 128-sample chunks in the [128, 2, N] slot layout).
     Host packs the 1024 nontrivial DFT rows (513 cos + 511 sin) so the
     contraction is exactly 1024; the frame matrix x[480t + n] is shipped
     pre-quantized to fp8 (final rel err ~2e-3, tolerance 2e-2).
  2. Square PSUM -> SBUF fp8 on ScalarE with scale 1/16 (sq = (X/16)^2).
  3. Mel projection in fp8 DoubleRow as well; mel filters are scaled by 64
     so their 0..0.02 range is representable; the net (1/256)*64 = 1/4
     factor is undone by the Ln activation's input scale of 4.
  4. Ln on ScalarE -> logmelT[t, m] bf16 (10/log(10) folded into R).
  5. Bilinear height-resize 1000 -> 1024 as a banded GEMM per half-batch
     (bf16 weights), overlapped with the next STFT group.
DMAs are merged to one descriptor per (batch, tt) input tile and one per
(batch, half) output tile -- the ~650 ns per-issue cost on the queue
engines otherwise serializes the head and tail.
"""

import numpy as np
import ml_dtypes

import concourse.bass as bass
import concourse.bacc as bacc
import concourse.tile as tile
import concourse.mybir as mybir
from concourse.bass_utils import run_bass_kernel_spmd

F32 = mybir.dt.float32
BF16 = mybir.dt.bfloat16
FP8 = mybir.dt.float8e4
NP_FP8 = ml_dtypes.float8_e4m3
DR = mybir.MatmulPerfMode.DoubleRow

N_FFT = 1024
HOP = 480
NB_MAX = 1000      # frames kept by the reference
N_MELS = 64
SPECW = 1024       # output height after resize
NBINS = 513
B, L = 32, 480000
NCORES = 8
BPC = B // NCORES  # batches per core
TFR = 1024         # padded frame count (frames >= 1000 are zeroed via R)
HT = TFR // 2
PAD = N_FFT // 2

# Set by test harness to collect a profile; harness default leaves it off.
TRACE = False
LAST_RESULTS = None


def _resize_blocks():
    """Banded resize matrix blocks, f32 coords matching the reference."""
    scale = np.float32((NB_MAX - 1) / (SPECW - 1))
    pos = np.arange(SPECW, dtype=np.float32) * scale
    h0 = np.clip(np.floor(pos).astype(np.int64), 0, NB_MAX - 1)
    frac = (pos - h0.astype(np.float32)).astype(np.float64)
    h1 = np.minimum(h0 + 1, NB_MAX - 1)
    c = 10.0 / np.log(10.0)
    r = np.zeros((TFR, SPECW), np.float64)
    cols = np.arange(SPECW)
    r[h0, cols] += (1.0 - frac) * c
    r[h1, cols] += frac * c
    blocks = np.zeros((8, 2, 128, 128), np.float32)
    qpairs = []
    for g in range(8):
        sub = r[:, g * 128:(g + 1) * 128]
        rows = np.nonzero(sub.any(axis=1))[0]
        qs = sorted({int(q) for q in rows // 128})
        assert 1 <= len(qs) <= 2, qs
        q0 = qs[0]
        q1 = qs[1] if len(qs) > 1 else min(q0 + 1, 7)
        blocks[g, 0] = sub[q0 * 128:(q0 + 1) * 128].astype(np.float32)
        blocks[g, 1] = sub[q1 * 128:(q1 + 1) * 128].astype(np.float32)
        qpairs.append((q0, q1))
    # [j(t-row), g, jj, h] so the whole bank loads as one [128, 8, 2, 128] DMA
    blk = np.ascontiguousarray(blocks.transpose(2, 0, 1, 3))
    return blk.astype(ml_dtypes.bfloat16), qpairs


_RBLOCKS, _QPAIRS = _resize_blocks()


def _build_bass():
    nc = bacc.Bacc("TRN2", target_bir_lowering=False, debug=False,
                   num_devices=NCORES)
    xt8 = nc.declare_dram_parameter("xt8", [BPC, 2, 128, 4, 2, HT], FP8,
                                    isOutput=False)
    wt = nc.declare_dram_parameter("wt", [128, 8, 4, 2, 128], FP8,
                                   isOutput=False)
    mexp = nc.declare_dram_parameter("mexp", [128, 4, 2, N_MELS], FP8,
                                     isOutput=False)
    rblk = nc.declare_dram_parameter("rblk", [128, 8, 2, 128], BF16,
                                     isOutput=False)
    out = nc.declare_dram_parameter("out", [BPC, 8, 128, N_MELS], F32,
                                    isOutput=True)

    with tile.TileContext(nc) as tc:
        with (
            tc.tile_pool(name="consts", bufs=1) as consts,
            tc.tile_pool(name="xt", bufs=3) as xpool,
            tc.tile_pool(name="sq", bufs=3) as sqpool,
            tc.tile_pool(name="lm", bufs=2) as lmpool,
            tc.tile_pool(name="ot", bufs=4) as otpool,
            tc.tile_pool(name="specp", bufs=4, space="PSUM") as specp,
            tc.tile_pool(name="melp", bufs=2, space="PSUM") as melp,
            tc.tile_pool(name="resp", bufs=2, space="PSUM") as resp,
        ):
            # m-major so the first chunk transferred covers the first
            # matmul groups; 4-split so the transfers run on parallel DMA
            # engines (a single descriptor chain sustains only ~115 GB/s)
            wt_t = consts.tile([128, 8, 4, 2, 128], FP8, tag="wt", name="wt")
            for c in range(4):
                nc.gpsimd.dma_start(out=wt_t[:, 2 * c:2 * c + 2],
                                    in_=wt[:, 2 * c:2 * c + 2])
            me_t = consts.tile([128, 4, 2, N_MELS], FP8, tag="me", name="me")
            rb_t = consts.tile([128, 8, 2, 128], BF16, tag="rb", name="rb")

            def load_late_consts():
                # issued after batch 0's frame DMA so they don't delay the
                # first STFT matmul (they're only needed ~10us in)
                nc.gpsimd.dma_start(out=me_t, in_=mexp[:])
                nc.gpsimd.dma_start(out=rb_t, in_=rblk[:])

            eps_t = consts.tile([128, 1], F32, tag="eps", name="eps")
            nc.vector.memset(eps_t, 1e-10)

            def issue_mel(sq_tiles, logmel, tt, s0=0, s1=4):
                mt = melp.tile([128, 4, N_MELS], F32, tag="mel", name="melpsum")
                for s in range(s0, s1):
                    for p in range(4):
                        nc.tensor.matmul(
                            mt[:, s - s0, :],
                            lhsT=sq_tiles[p][:, :, s * 128:(s + 1) * 128],
                            rhs=me_t[:, p],
                            start=(p == 0),
                            stop=(p == 3),
                            perf_mode=DR,
                        )
                nc.scalar.activation(
                    out=logmel[:, tt * 4 + s0:tt * 4 + s1, :],
                    in_=mt[:, 0:s1 - s0, :],
                    func=mybir.ActivationFunctionType.Ln,
                    bias=eps_t,
                    scale=4.0,
                )

            def issue_resize(logmel, b, g0, g1, eng):
                rp = resp.tile([128, 4, N_MELS], F32, tag="res", name="respsum")
                for gg in range(g1 - g0):
                    g = g0 + gg
                    q0, q1 = _QPAIRS[g]
                    nc.tensor.matmul(rp[:, gg, :], lhsT=rb_t[:, g, 0],
                                     rhs=logmel[:, q0, :],
                                     start=True, stop=False)
                    nc.tensor.matmul(rp[:, gg, :], lhsT=rb_t[:, g, 1],
                                     rhs=logmel[:, q1, :],
                                     start=False, stop=True)
                ot = otpool.tile([128, 4, N_MELS], F32, tag="ot", name="ot")
                nc.vector.tensor_copy(out=ot[:, 0:g1 - g0, :],
                                      in_=rp[:, 0:g1 - g0, :])
                eng.dma_start(
                    out=out[b, g0:g1].transpose([1, 0, 2]),
                    in_=ot[:, 0:g1 - g0, :],
                )

            def load_xt(t, b, tt):
                if b == 0 and tt == 0:
                    # head-latency critical: t-halved and p-split (8 x
                    # 128 KB) so the first 256-column cut can start after
                    # only half the tile has landed
                    for th in range(2):
                        for p in range(4):
                            sl = slice(th * 256, (th + 1) * 256)
                            nc.sync.dma_start(out=t[:, p, :, sl],
                                              in_=xt8[b, tt, :, p, :, sl])
                elif b == 0:
                    for p in range(4):
                        nc.sync.dma_start(out=t[:, p], in_=xt8[b, tt, :, p])
                else:
                    nc.sync.dma_start(out=t, in_=xt8[b, tt])

            pending = []
            for b in range(BPC):
                xt = []
                for tt in range(2):
                    t = xpool.tile([128, 4, 2, HT], FP8, tag=f"xt{tt}",
                                   name=f"xt{tt}")
                    # batch 0: defer the tt=1 load until after the first
                    # group's matmuls are emitted -- a matmul's wait is
                    # coarsened to the queue's last-issued DMA, so issuing
                    # it here would stall the first matmul on it
                    if not (b == 0 and tt == 1):
                        load_xt(t, b, tt)
                    xt.append(t)
                logmel = lmpool.tile([128, 8, N_MELS], BF16, tag="lm",
                                     name="logmel")
                for tt in range(2):
                    last = (b == BPC - 1 and tt == 1)
                    first = (b == 0 and tt == 0)
                    # frames >= NB_MAX are junk the resize matrix zeroes out,
                    # so tt=1 only computes 488 of 512 columns; the first and
                    # last groups split in half so the head DMA respectively
                    # the trailing squares overlap the matmul stream
                    end = 512 if tt == 0 else NB_MAX - HT
                    cuts = (((0, 256), (256, end)) if (last or first)
                            else ((0, end),))
                    sq_tiles = []
                    for m in range(8):
                        if m % 2 == 0:
                            sq = sqpool.tile([128, 2, 512], FP8,
                                             tag=f"sq{m // 2}",
                                             name=f"sq{m // 2}")
                            sq_tiles.append(sq)
                            if b == 0 and tt == 1:
                                # first use of this ring instance: keep the
                                # uncomputed junk-frame columns finite (NaN
                                # would propagate through 0 * NaN in the
                                # resize matmul)
                                nc.vector.memset(sq[:, :, end:], 0)
                    for lo, hi in cuts:
                        for m in range(8):
                            ps = specp.tile([128, 512], F32, tag="spec",
                                            name="specpsum")
                            for p in range(4):
                                nc.tensor.matmul(
                                    ps[:, lo:hi],
                                    lhsT=wt_t[:, m, p],
                                    rhs=xt[tt][:, p, :, lo:hi],
                                    start=(p == 0),
                                    stop=(p == 3),
                                    perf_mode=DR,
                                )
                            nc.scalar.activation(
                                out=sq_tiles[m // 2][:, m % 2, lo:hi],
                                in_=ps[:, lo:hi],
                                func=mybir.ActivationFunctionType.Square,
                                bias=0.0,
                                scale=0.0625,
                            )
                        if lo == 0:
                            for fn in pending:
                                fn()
                            pending = []
                            if b == 0 and tt == 0:
                                load_xt(xt[1], 0, 1)
                                load_late_consts()
                        elif last:
                            # overlap the final tail: mel/Ln/resize for the
                            # first half of the last group run under the
                            # second half's matmul stream
                            issue_mel(sq_tiles, logmel, tt, 0, 2)
                            issue_resize(logmel, b, 4, 6, nc.gpsimd)
                    if last:
                        issue_mel(sq_tiles, logmel, tt, 2, 4)
                        issue_resize(logmel, b, 6, 8, nc.scalar)
                        pending = []
                    else:
                        pending = [
                            (lambda s=sq_tiles, l=logmel, t=tt:
                                issue_mel(s, l, t)),
                            (lambda l=logmel, bb=b, h=tt:
                                issue_resize(l, bb, h * 4, h * 4 + 4,
                                             nc.gpsimd)),
                        ]
            for fn in pending:
                fn()
    return nc


def _host_prep(waveform, stft_weights, mel_filters):
    wv = np.ascontiguousarray(waveform, dtype=np.float32)
    xp = np.pad(wv, ((0, 0), (PAD, PAD)), mode="reflect")  # [B, 481024]
    need = HOP * (TFR - 1) + N_FFT  # max index reached by a frame + 1
    xz = np.zeros((B, need), NP_FP8)
    xz[:, : xp.shape[1]] = xp.astype(NP_FP8)
    sb = xz.strides[0]
    # xt8[b, tt, j, p, s, t'] = x[480(512tt + t') + 256p + 128s + j] --
    # one contiguous-per-partition DMA per (b, tt) in the DoubleRow
    # [128, 2, N] slot layout.
    xt8 = np.ascontiguousarray(np.lib.stride_tricks.as_strided(
        xz, shape=(B, 2, 128, 4, 2, HT),
        strides=(sb, HOP * HT, 1, 256, 128, HOP)))

    w = np.ascontiguousarray(stft_weights, dtype=np.float32)  # [1026, 1024]
    rows = list(range(0, NBINS)) + list(range(NBINS + 1, NBINS + 512))
    assert len(rows) == 1024
    wp = w[rows]                                   # [1024 packed bins, 1024 n]
    # wt[j, m, p, s, k'] = W[128m + k', 256p + 128s + j]
    wt = wp.T.reshape(4, 2, 128, 8, 128).transpose(2, 3, 0, 1, 4)
    wt = np.ascontiguousarray(wt).astype(NP_FP8)

    mf = np.ascontiguousarray(mel_filters, dtype=np.float32)  # [513, 64]
    f_of_i = np.array([i if i < NBINS else i - 512 for i in range(1024)])
    # mexp[j, p, s, m] = 64 * mf[bin(256p + 128s + j), m]; the 64 and the
    # 1/256 from sq = (X/16)^2 are undone by the Ln input scale of 4.
    mexp = (64.0 * mf[f_of_i]).reshape(4, 2, 128, N_MELS)
    mexp = np.ascontiguousarray(mexp.transpose(2, 0, 1, 3)).astype(NP_FP8)
    return xt8, wt, mexp


def kernel(waveform, stft_weights, mel_filters):
    global LAST_RESULTS
    xt8, wt, mexp = _host_prep(waveform, stft_weights, mel_filters)
    nc = _build_bass()
    in_maps = []
    for i in range(NCORES):
        in_maps.append({
            "xt8": np.ascontiguousarray(xt8[i * BPC:(i + 1) * BPC]),
            "wt": wt,
            "mexp": mexp,
            "rblk": _RBLOCKS,
        })
    nc.compile()
    res = run_bass_kernel_spmd(nc, in_maps, list(range(NCORES)), trace=TRACE)
    LAST_RESULTS = res
    out = np.concatenate([r["out"] for r in res.results], axis=0)
    return out.reshape(B, 1, SPECW, N_MELS).astype(np.float32)


# revision 36
# speedup vs baseline: 1.0134x; 1.0134x over previous
"""Trainium2 Bass kernel for nn_HFMelSpectrogram.

Pipeline (per core, 4 batches of the 32-batch waveform):
  1. STFT-as-GEMM in fp8e4m3 with DoubleRow perf mode: the PE array is
     virtualized to 128x256, so the 1024-deep contraction takes 4 matmuls
     (each pairs two 128-sample chunks in the [128, 2, N] slot layout).
     Host packs the 1024 nontrivial DFT rows (513 cos + 511 sin) so the
     contraction is exactly 1024; the frame matrix x[480t + n] is shipped
     pre-quantized to fp8 (final rel err ~2e-3, tolerance 2e-2).
  2. Square PSUM -> SBUF fp8 on ScalarE with scale 1/16 (sq = (X/16)^2).
  3. Mel projection in fp8 DoubleRow as well; mel filters are scaled by 64
     so their 0..0.02 range is representable; the net (1/256)*64 = 1/4
     factor is undone by the Ln activation's input scale of 4.
  4. Ln on ScalarE -> logmelT[t, m] bf16 (10/log(10) folded into R).
  5. Bilinear height-resize 1000 -> 1024 as a banded GEMM per half-batch
     (bf16 weights), overlapped with the next STFT group.
DMAs are merged to one descriptor per (batch, tt) input tile and one per
(batch, half) output tile -- the ~650 ns per-issue cost on the queue
engines otherwise serializes the head and tail.
"""

import numpy as np
import ml_dtypes

import concourse.bass as bass
import concourse.bacc as bacc
import concourse.tile as tile
import concourse.mybir as mybir
from concourse.bass_utils import run_bass_kernel_spmd

F32 = mybir.dt.float32
BF16 = mybir.dt.bfloat16
FP8 = mybir.dt.float8e4
NP_FP8 = ml_dtypes.float8_e4m3
DR = mybir.MatmulPerfMode.DoubleRow

N_FFT = 1024
HOP = 480
NB_MAX = 1000      # frames kept by the reference
N_MELS = 64
SPECW = 1024       # output height after resize
NBINS = 513
B, L = 32, 480000
NCORES = 8
BPC = B // NCORES  # batches per core
TFR = 1024         # padded frame count (frames >= 1000 are zeroed via R)
HT = TFR // 2
PAD = N_FFT // 2

# Set by test harness to collect a profile; harness default leaves it off.
TRACE = False
LAST_RESULTS = None


def _resize_blocks():
    """Banded resize matrix blocks, f32 coords matching the reference."""
    scale = np.float32((NB_MAX - 1) / (SPECW - 1))
    pos = np.arange(SPECW, dtype=np.float32) * scale
    h0 = np.clip(np.floor(pos).astype(np.int64), 0, NB_MAX - 1)
    frac = (pos - h0.astype(np.float32)).astype(np.float64)
    h1 = np.minimum(h0 + 1, NB_MAX - 1)
    c = 10.0 / np.log(10.0)
    r = np.zeros((TFR, SPECW), np.float64)
    cols = np.arange(SPECW)
    r[h0, cols] += (1.0 - frac) * c
    r[h1, cols] += frac * c
    blocks = np.zeros((8, 2, 128, 128), np.float32)
    qpairs = []
    for g in range(8):
        sub = r[:, g * 128:(g + 1) * 128]
        rows = np.nonzero(sub.any(axis=1))[0]
        qs = sorted({int(q) for q in rows // 128})
        assert 1 <= len(qs) <= 2, qs
        q0 = qs[0]
        q1 = qs[1] if len(qs) > 1 else min(q0 + 1, 7)
        blocks[g, 0] = sub[q0 * 128:(q0 + 1) * 128].astype(np.float32)
        blocks[g, 1] = sub[q1 * 128:(q1 + 1) * 128].astype(np.float32)
        qpairs.append((q0, q1))
    # [j(t-row), g, jj, h] so the whole bank loads as one [128, 8, 2, 128] DMA
    blk = np.ascontiguousarray(blocks.transpose(2, 0, 1, 3))
    return blk.astype(ml_dtypes.bfloat16), qpairs


_RBLOCKS, _QPAIRS = _resize_blocks()


def _build_bass():
    nc = bacc.Bacc("TRN2", target_bir_lowering=False, debug=False,
                   num_devices=NCORES)
    xt8 = nc.declare_dram_parameter("xt8", [BPC, 2, 128, 4, 2, HT], FP8,
                                    isOutput=False)
    wt = nc.declare_dram_parameter("wt", [128, 4, 2, 1024], FP8, isOutput=False)
    mexp = nc.declare_dram_parameter("mexp", [128, 4, 2, N_MELS], FP8,
                                     isOutput=False)
    rblk = nc.declare_dram_parameter("rblk", [128, 8, 2, 128], BF16,
                                     isOutput=False)
    out = nc.declare_dram_parameter("out", [BPC, 8, 128, N_MELS], F32,
                                    isOutput=True)

    with tile.TileContext(nc) as tc:
        with (
            tc.tile_pool(name="consts", bufs=1) as consts,
            tc.tile_pool(name="xt", bufs=3) as xpool,
            tc.tile_pool(name="sq", bufs=3) as sqpool,
            tc.tile_pool(name="lm", bufs=2) as lmpool,
            tc.tile_pool(name="ot", bufs=4) as otpool,
            tc.tile_pool(name="specp", bufs=4, space="PSUM") as specp,
            tc.tile_pool(name="melp", bufs=2, space="PSUM") as melp,
            tc.tile_pool(name="resp", bufs=2, space="PSUM") as resp,
        ):
            wt_t = consts.tile([128, 4, 2, 1024], FP8, tag="wt", name="wt")
            for p in range(4):
                # p-split so the transfers run on parallel DMA engines
                # (a single descriptor chain sustains only ~115 GB/s)
                nc.gpsimd.dma_start(out=wt_t[:, p], in_=wt[:, p])
            me_t = consts.tile([128, 4, 2, N_MELS], FP8, tag="me", name="me")
            rb_t = consts.tile([128, 8, 2, 128], BF16, tag="rb", name="rb")

            def load_late_consts():
                # issued after batch 0's frame DMA so they don't delay the
                # first STFT matmul (they're only needed ~10us in)
                nc.gpsimd.dma_start(out=me_t, in_=mexp[:])
                nc.gpsimd.dma_start(out=rb_t, in_=rblk[:])

            eps_t = consts.tile([128, 1], F32, tag="eps", name="eps")
            nc.vector.memset(eps_t, 1e-10)

            def issue_mel(sq_tiles, logmel, tt, s0=0, s1=4):
                mt = melp.tile([128, 4, N_MELS], F32, tag="mel", name="melpsum")
                for s in range(s0, s1):
                    for p in range(4):
                        nc.tensor.matmul(
                            mt[:, s - s0, :],
                            lhsT=sq_tiles[p][:, :, s * 128:(s + 1) * 128],
                            rhs=me_t[:, p],
                            start=(p == 0),
                            stop=(p == 3),
                            perf_mode=DR,
                        )
                nc.scalar.activation(
                    out=logmel[:, tt * 4 + s0:tt * 4 + s1, :],
                    in_=mt[:, 0:s1 - s0, :],
                    func=mybir.ActivationFunctionType.Ln,
                    bias=eps_t,
                    scale=4.0,
                )

            def issue_resize(logmel, b, g0, g1, eng):
                rp = resp.tile([128, 4, N_MELS], F32, tag="res", name="respsum")
                for gg in range(g1 - g0):
                    g = g0 + gg
                    q0, q1 = _QPAIRS[g]
                    nc.tensor.matmul(rp[:, gg, :], lhsT=rb_t[:, g, 0],
                                     rhs=logmel[:, q0, :],
                                     start=True, stop=False)
                    nc.tensor.matmul(rp[:, gg, :], lhsT=rb_t[:, g, 1],
                                     rhs=logmel[:, q1, :],
                                     start=False, stop=True)
                ot = otpool.tile([128, 4, N_MELS], F32, tag="ot", name="ot")
                nc.vector.tensor_copy(out=ot[:, 0:g1 - g0, :],
                                      in_=rp[:, 0:g1 - g0, :])
                eng.dma_start(
                    out=out[b, g0:g1].transpose([1, 0, 2]),
                    in_=ot[:, 0:g1 - g0, :],
                )

            def load_xt(t, b, tt):
                if b == 0:
                    # head-latency critical: 4-way p-split so the transfers
                    # run on parallel DMA engines
                    for p in range(4):
                        nc.sync.dma_start(out=t[:, p], in_=xt8[b, tt, :, p])
                else:
                    nc.sync.dma_start(out=t, in_=xt8[b, tt])

            pending = []
            for b in range(BPC):
                xt = []
                for tt in range(2):
                    t = xpool.tile([128, 4, 2, HT], FP8, tag=f"xt{tt}",
                                   name=f"xt{tt}")
                    load_xt(t, b, tt)
                    xt.append(t)
                if b == 0:
                    load_late_consts()
                logmel = lmpool.tile([128, 8, N_MELS], BF16, tag="lm",
                                     name="logmel")
                for tt in range(2):
                    last = (b == BPC - 1 and tt == 1)
                    # frames >= NB_MAX are junk the resize matrix zeroes out,
                    # so tt=1 only computes 488 of 512 columns; the last
                    # group additionally splits so its trailing squares
                    # overlap the second half's matmul stream
                    end = 512 if tt == 0 else NB_MAX - HT
                    cuts = ((0, 256), (256, end)) if last else ((0, end),)
                    sq_tiles = []
                    for m in range(8):
                        if m % 2 == 0:
                            sq = sqpool.tile([128, 2, 512], FP8,
                                             tag=f"sq{m // 2}",
                                             name=f"sq{m // 2}")
                            sq_tiles.append(sq)
                            if b == 0 and tt == 1:
                                # first use of this ring instance: keep the
                                # uncomputed junk-frame columns finite (NaN
                                # would propagate through 0 * NaN in the
                                # resize matmul)
                                nc.vector.memset(sq[:, :, end:], 0)
                    for lo, hi in cuts:
                        for m in range(8):
                            ps = specp.tile([128, 512], F32, tag="spec",
                                            name="specpsum")
                            for p in range(4):
                                nc.tensor.matmul(
                                    ps[:, lo:hi],
                                    lhsT=wt_t[:, p, :, m * 128:(m + 1) * 128],
                                    rhs=xt[tt][:, p, :, lo:hi],
                                    start=(p == 0),
                                    stop=(p == 3),
                                    perf_mode=DR,
                                )
                            nc.scalar.activation(
                                out=sq_tiles[m // 2][:, m % 2, lo:hi],
                                in_=ps[:, lo:hi],
                                func=mybir.ActivationFunctionType.Square,
                                bias=0.0,
                                scale=0.0625,
                            )
                        if lo == 0:
                            for fn in pending:
                                fn()
                            pending = []
                        elif last:
                            # overlap the final tail: mel/Ln/resize for the
                            # first half of the last group run under the
                            # second half's matmul stream
                            issue_mel(sq_tiles, logmel, tt, 0, 2)
                            issue_resize(logmel, b, 4, 6, nc.gpsimd)
                    if last:
                        issue_mel(sq_tiles, logmel, tt, 2, 4)
                        issue_resize(logmel, b, 6, 8, nc.scalar)
                        pending = []
                    else:
                        pending = [
                            (lambda s=sq_tiles, l=logmel, t=tt:
                                issue_mel(s, l, t)),
                            (lambda l=logmel, bb=b, h=tt:
                                issue_resize(l, bb, h * 4, h * 4 + 4,
                                             nc.gpsimd)),
                        ]
            for fn in pending:
                fn()
    return nc


def _host_prep(waveform, stft_weights, mel_filters):
    wv = np.ascontiguousarray(waveform, dtype=np.float32)
    xp = np.pad(wv, ((0, 0), (PAD, PAD)), mode="reflect")  # [B, 481024]
    need = HOP * (TFR - 1) + N_FFT  # max index reached by a frame + 1
    xz = np.zeros((B, need), NP_FP8)
    xz[:, : xp.shape[1]] = xp.astype(NP_FP8)
    sb = xz.strides[0]
    # xt8[b, tt, j, p, s, t'] = x[480(512tt + t') + 256p + 128s + j] --
    # one contiguous-per-partition DMA per (b, tt) in the DoubleRow
    # [128, 2, N] slot layout.
    xt8 = np.ascontiguousarray(np.lib.stride_tricks.as_strided(
        xz, shape=(B, 2, 128, 4, 2, HT),
        strides=(sb, HOP * HT, 1, 256, 128, HOP)))

    w = np.ascontiguousarray(stft_weights, dtype=np.float32)  # [1026, 1024]
    rows = list(range(0, NBINS)) + list(range(NBINS + 1, NBINS + 512))
    assert len(rows) == 1024
    wp = w[rows]                                   # [1024 packed bins, 1024 n]
    # wt[j, p, s, k] = W[k, 256p + 128s + j]
    wt = wp.T.reshape(4, 2, 128, 1024).transpose(2, 0, 1, 3)
    wt = np.ascontiguousarray(wt).astype(NP_FP8)

    mf = np.ascontiguousarray(mel_filters, dtype=np.float32)  # [513, 64]
    f_of_i = np.array([i if i < NBINS else i - 512 for i in range(1024)])
    # mexp[j, p, s, m] = 64 * mf[bin(256p + 128s + j), m]; the 64 and the
    # 1/256 from sq = (X/16)^2 are undone by the Ln input scale of 4.
    mexp = (64.0 * mf[f_of_i]).reshape(4, 2, 128, N_MELS)
    mexp = np.ascontiguousarray(mexp.transpose(2, 0, 1, 3)).astype(NP_FP8)
    return xt8, wt, mexp


def kernel(waveform, stft_weights, mel_filters):
    global LAST_RESULTS
    xt8, wt, mexp = _host_prep(waveform, stft_weights, mel_filters)
    nc = _build_bass()
    in_maps = []
    for i in range(NCORES):
        in_maps.append({
            "xt8": np.ascontiguousarray(xt8[i * BPC:(i + 1) * BPC]),
            "wt": wt,
            "mexp": mexp,
            "rblk": _RBLOCKS,
        })
    nc.compile()
    res = run_bass_kernel_spmd(nc, in_maps, list(range(NCORES)), trace=TRACE)
    LAST_RESULTS = res
    out = np.concatenate([r["out"] for r in res.results], axis=0)
    return out.reshape(B, 1, SPECW, N_MELS).astype(np.float32)


# revision 38
# speedup vs baseline: 1.0143x; 1.0008x over previous
"""Trainium2 Bass kernel for nn_HFMelSpectrogram.

Pipeline (per core, 4 batches of the 32-batch waveform):
  1. STFT-as-GEMM in fp8e4m3 with DoubleRow perf mode: the PE array is
     virtualized to 128x256, so the 1024-deep contraction takes 4 matmuls
     (each pairs two 128-sample chunks in the [128, 2, N] slot layout).
     Host packs the 1024 nontrivial DFT rows (513 cos + 511 sin) so the
     contraction is exactly 1024; the frame matrix x[480t + n] is shipped
     pre-quantized to fp8 (final rel err ~2e-3, tolerance 2e-2).
  2. Square PSUM -> SBUF fp8 on ScalarE with scale 1/16 (sq = (X/16)^2).
  3. Mel projection in fp8 DoubleRow as well; mel filters are scaled by 64
     so their 0..0.02 range is representable; the net (1/256)*64 = 1/4
     factor is undone by the Ln activation's input scale of 4.
  4. Ln on ScalarE -> logmelT[t, m] bf16 (10/log(10) folded into R).
  5. Bilinear height-resize 1000 -> 1024 as a banded GEMM per half-batch
     (bf16 weights), overlapped with the next STFT group.
DMAs are merged to one descriptor per (batch, tt) input tile and one per
(batch, half) output tile -- the ~650 ns per-issue cost on the queue
engines otherwise serializes the head and tail.
"""

import numpy as np
import ml_dtypes

import concourse.bass as bass
import concourse.bacc as bacc
import concourse.tile as tile
import concourse.mybir as mybir
from concourse.bass_utils import run_bass_kernel_spmd

F32 = mybir.dt.float32
BF16 = mybir.dt.bfloat16
FP8 = mybir.dt.float8e4
NP_FP8 = ml_dtypes.float8_e4m3
DR = mybir.MatmulPerfMode.DoubleRow

N_FFT = 1024
HOP = 480
NB_MAX = 1000      # frames kept by the reference
N_MELS = 64
SPECW = 1024       # output height after resize
NBINS = 513
B, L = 32, 480000
NCORES = 8
BPC = B // NCORES  # batches per core
TFR = 1024         # padded frame count (frames >= 1000 are zeroed via R)
HT = TFR // 2
PAD = N_FFT // 2

# Set by test harness to collect a profile; harness default leaves it off.
TRACE = False
LAST_RESULTS = None


def _resize_blocks():
    """Banded resize matrix blocks, f32 coords matching the reference."""
    scale = np.float32((NB_MAX - 1) / (SPECW - 1))
    pos = np.arange(SPECW, dtype=np.float32) * scale
    h0 = np.clip(np.floor(pos).astype(np.int64), 0, NB_MAX - 1)
    frac = (pos - h0.astype(np.float32)).astype(np.float64)
    h1 = np.minimum(h0 + 1, NB_MAX - 1)
    c = 10.0 / np.log(10.0)
    r = np.zeros((TFR, SPECW), np.float64)
    cols = np.arange(SPECW)
    r[h0, cols] += (1.0 - frac) * c
    r[h1, cols] += frac * c
    blocks = np.zeros((8, 2, 128, 128), np.float32)
    qpairs = []
    for g in range(8):
        sub = r[:, g * 128:(g + 1) * 128]
        rows = np.nonzero(sub.any(axis=1))[0]
        qs = sorted({int(q) for q in rows // 128})
        assert 1 <= len(qs) <= 2, qs
        q0 = qs[0]
        q1 = qs[1] if len(qs) > 1 else min(q0 + 1, 7)
        blocks[g, 0] = sub[q0 * 128:(q0 + 1) * 128].astype(np.float32)
        blocks[g, 1] = sub[q1 * 128:(q1 + 1) * 128].astype(np.float32)
        qpairs.append((q0, q1))
    # [j(t-row), g, jj, h] so the whole bank loads as one [128, 8, 2, 128] DMA
    blk = np.ascontiguousarray(blocks.transpose(2, 0, 1, 3))
    return blk.astype(ml_dtypes.bfloat16), qpairs


_RBLOCKS, _QPAIRS = _resize_blocks()


def _build_bass():
    nc = bacc.Bacc("TRN2", target_bir_lowering=False, debug=False,
                   num_devices=NCORES)
    xt8 = nc.declare_dram_parameter("xt8", [BPC, 2, 128, 4, 2, HT], FP8,
                                    isOutput=False)
    xth = nc.declare_dram_parameter("xth", [2, 128, 4, 2, HT // 2], FP8,
                                    isOutput=False)
    wt = nc.declare_dram_parameter("wt", [128, 8, 4, 2, 128], FP8,
                               isOutput=False)
    mexp = nc.declare_dram_parameter("mexp", [128, 4, 2, N_MELS], FP8,
                                     isOutput=False)
    rblk = nc.declare_dram_parameter("rblk", [128, 8, 2, 128], BF16,
                                     isOutput=False)
    out = nc.declare_dram_parameter("out", [BPC, 8, 128, N_MELS], F32,
                                    isOutput=True)

    with tile.TileContext(nc) as tc:
        with (
            tc.tile_pool(name="consts", bufs=1) as consts,
            tc.tile_pool(name="xt", bufs=3) as xpool,
            tc.tile_pool(name="sq", bufs=3) as sqpool,
            tc.tile_pool(name="lm", bufs=2) as lmpool,
            tc.tile_pool(name="ot", bufs=4) as otpool,
            tc.tile_pool(name="specp", bufs=4, space="PSUM") as specp,
            tc.tile_pool(name="melp", bufs=2, space="PSUM") as melp,
            tc.tile_pool(name="resp", bufs=2, space="PSUM") as resp,
        ):
            # m-major so the first chunk transferred covers the first
            # matmul groups; 4-split so the transfers run on parallel DMA
            # engines (a single descriptor chain sustains only ~115 GB/s)
            wt_t = consts.tile([128, 8, 4, 2, 128], FP8, tag="wt", name="wt")
            for c in range(4):
                nc.gpsimd.dma_start(out=wt_t[:, 2 * c:2 * c + 2],
                                    in_=wt[:, 2 * c:2 * c + 2])
            me_t = consts.tile([128, 4, 2, N_MELS], FP8, tag="me", name="me")
            rb_t = consts.tile([128, 8, 2, 128], BF16, tag="rb", name="rb")

            def load_late_consts():
                # issued after batch 0's frame DMA so they don't delay the
                # first STFT matmul (they're only needed ~10us in)
                nc.gpsimd.dma_start(out=me_t, in_=mexp[:])
                nc.gpsimd.dma_start(out=rb_t, in_=rblk[:])

            eps_t = consts.tile([128, 1], F32, tag="eps", name="eps")
            nc.vector.memset(eps_t, 1e-10)

            def issue_mel(sq_tiles, logmel, tt, s0=0, s1=4):
                mt = melp.tile([128, 4, N_MELS], F32, tag="mel", name="melpsum")
                for s in range(s0, s1):
                    for p in range(4):
                        nc.tensor.matmul(
                            mt[:, s - s0, :],
                            lhsT=sq_tiles[p][:, :, s * 128:(s + 1) * 128],
                            rhs=me_t[:, p],
                            start=(p == 0),
                            stop=(p == 3),
                            perf_mode=DR,
                        )
                nc.scalar.activation(
                    out=logmel[:, tt * 4 + s0:tt * 4 + s1, :],
                    in_=mt[:, 0:s1 - s0, :],
                    func=mybir.ActivationFunctionType.Ln,
                    bias=eps_t,
                    scale=4.0,
                )

            def issue_resize(logmel, b, g0, g1, eng):
                rp = resp.tile([128, 4, N_MELS], F32, tag="res", name="respsum")
                for gg in range(g1 - g0):
                    g = g0 + gg
                    q0, q1 = _QPAIRS[g]
                    nc.tensor.matmul(rp[:, gg, :], lhsT=rb_t[:, g, 0],
                                     rhs=logmel[:, q0, :],
                                     start=True, stop=False)
                    nc.tensor.matmul(rp[:, gg, :], lhsT=rb_t[:, g, 1],
                                     rhs=logmel[:, q1, :],
                                     start=False, stop=True)
                ot = otpool.tile([128, 4, N_MELS], F32, tag="ot", name="ot")
                nc.vector.tensor_copy(out=ot[:, 0:g1 - g0, :],
                                      in_=rp[:, 0:g1 - g0, :])
                eng.dma_start(
                    out=out[b, g0:g1].transpose([1, 0, 2]),
                    in_=ot[:, 0:g1 - g0, :],
                )

            def load_xt(t, b, tt):
                if b == 0 and tt == 0:
                    # head-latency critical: th-major duplicate layout so
                    # each 256-column half is one contiguous transfer and
                    # the first cut starts after half the data
                    for th in range(2):
                        nc.sync.dma_start(out=t[:, th], in_=xth[th])
                elif b == 0:
                    for p in range(4):
                        nc.sync.dma_start(out=t[:, p], in_=xt8[b, tt, :, p])
                else:
                    nc.sync.dma_start(out=t, in_=xt8[b, tt])

            pending = []
            for b in range(BPC):
                xt = []
                for tt in range(2):
                    shape = ([128, 2, 4, 2, HT // 2] if b == 0 and tt == 0
                             else [128, 4, 2, HT])
                    t = xpool.tile(shape, FP8, tag=f"xt{tt}", name=f"xt{tt}")
                    load_xt(t, b, tt)
                    xt.append(t)
                if b == 0:
                    load_late_consts()
                logmel = lmpool.tile([128, 8, N_MELS], BF16, tag="lm",
                                     name="logmel")
                for tt in range(2):
                    last = (b == BPC - 1 and tt == 1)
                    first = (b == 0 and tt == 0)
                    # frames >= NB_MAX are junk the resize matrix zeroes out,
                    # so tt=1 only computes 488 of 512 columns; the first and
                    # last groups split in half so the head DMA respectively
                    # the trailing squares overlap the matmul stream
                    end = 512 if tt == 0 else NB_MAX - HT
                    cuts = (((0, 256), (256, end)) if (last or first)
                            else ((0, end),))
                    sq_tiles = []
                    for m in range(8):
                        if m % 2 == 0:
                            sq = sqpool.tile([128, 2, 512], FP8,
                                             tag=f"sq{m // 2}",
                                             name=f"sq{m // 2}")
                            sq_tiles.append(sq)
                            if b == 0 and tt == 1:
                                # first use of this ring instance: keep the
                                # uncomputed junk-frame columns finite (NaN
                                # would propagate through 0 * NaN in the
                                # resize matmul)
                                nc.vector.memset(sq[:, :, end:], 0)
                    for lo, hi in cuts:
                        for m in range(8):
                            ps = specp.tile([128, 512], F32, tag="spec",
                                            name="specpsum")
                            for p in range(4):
                                nc.tensor.matmul(
                                    ps[:, lo:hi],
                                    lhsT=wt_t[:, m, p],
                                    rhs=(xt[tt][:, lo // 256, p]
                                         if b == 0 and tt == 0
                                         else xt[tt][:, p, :, lo:hi]),
                                    start=(p == 0),
                                    stop=(p == 3),
                                    perf_mode=DR,
                                )
                            nc.scalar.activation(
                                out=sq_tiles[m // 2][:, m % 2, lo:hi],
                                in_=ps[:, lo:hi],
                                func=mybir.ActivationFunctionType.Square,
                                bias=0.0,
                                scale=0.0625,
                            )
                        if lo == 0:
                            for fn in pending:
                                fn()
                            pending = []
                        elif last:
                            # overlap the final tail: mel/Ln/resize for the
                            # first half of the last group run under the
                            # second half's matmul stream
                            issue_mel(sq_tiles, logmel, tt, 0, 2)
                            issue_resize(logmel, b, 4, 6, nc.gpsimd)
                    if last:
                        issue_mel(sq_tiles, logmel, tt, 2, 4)
                        issue_resize(logmel, b, 6, 8, nc.scalar)
                        pending = []
                    else:
                        pending = [
                            (lambda s=sq_tiles, l=logmel, t=tt:
                                issue_mel(s, l, t)),
                            (lambda l=logmel, bb=b, h=tt:
                                issue_resize(l, bb, h * 4, h * 4 + 4,
                                             nc.gpsimd)),
                        ]
            for fn in pending:
                fn()
    return nc


def _host_prep(waveform, stft_weights, mel_filters):
    wv = np.ascontiguousarray(waveform, dtype=np.float32)
    xp = np.pad(wv, ((0, 0), (PAD, PAD)), mode="reflect")  # [B, 481024]
    need = HOP * (TFR - 1) + N_FFT  # max index reached by a frame + 1
    xz = np.zeros((B, need), NP_FP8)
    xz[:, : xp.shape[1]] = xp.astype(NP_FP8)
    sb = xz.strides[0]
    # xt8[b, tt, j, p, s, t'] = x[480(512tt + t') + 256p + 128s + j] --
    # one contiguous-per-partition DMA per (b, tt) in the DoubleRow
    # [128, 2, N] slot layout.
    xt8 = np.ascontiguousarray(np.lib.stride_tricks.as_strided(
        xz, shape=(B, 2, 128, 4, 2, HT),
        strides=(sb, HOP * HT, 1, 256, 128, HOP)))

    w = np.ascontiguousarray(stft_weights, dtype=np.float32)  # [1026, 1024]
    rows = list(range(0, NBINS)) + list(range(NBINS + 1, NBINS + 512))
    assert len(rows) == 1024
    wp = w[rows]                                   # [1024 packed bins, 1024 n]
    # wt[j, m, p, s, k'] = W[128m + k', 256p + 128s + j]
    wt = wp.T.reshape(4, 2, 128, 8, 128).transpose(2, 3, 0, 1, 4)
    wt = np.ascontiguousarray(wt).astype(NP_FP8)

    # th-major contiguous duplicate of each core's first (b0, tt0) tile
    xth = np.ascontiguousarray(np.lib.stride_tricks.as_strided(
        xz, shape=(B, 2, 128, 4, 2, HT // 2),
        strides=(sb, HOP * (HT // 2), 1, 256, 128, HOP)))

    mf = np.ascontiguousarray(mel_filters, dtype=np.float32)  # [513, 64]
    f_of_i = np.array([i if i < NBINS else i - 512 for i in range(1024)])
    # mexp[j, p, s, m] = 64 * mf[bin(256p + 128s + j), m]; the 64 and the
    # 1/256 from sq = (X/16)^2 are undone by the Ln input scale of 4.
    mexp = (64.0 * mf[f_of_i]).reshape(4, 2, 128, N_MELS)
    mexp = np.ascontiguousarray(mexp.transpose(2, 0, 1, 3)).astype(NP_FP8)
    return xt8, xth, wt, mexp


def kernel(waveform, stft_weights, mel_filters):
    global LAST_RESULTS
    xt8, xth, wt, mexp = _host_prep(waveform, stft_weights, mel_filters)
    nc = _build_bass()
    in_maps = []
    for i in range(NCORES):
        in_maps.append({
            "xt8": np.ascontiguousarray(xt8[i * BPC:(i + 1) * BPC]),
            "xth": np.ascontiguousarray(xth[i * BPC]),
            "wt": wt,
            "mexp": mexp,
            "rblk": _RBLOCKS,
        })
    nc.compile()
    res = run_bass_kernel_spmd(nc, in_maps, list(range(NCORES)), trace=TRACE)
    LAST_RESULTS = res
    out = np.concatenate([r["out"] for r in res.results], axis=0)
    return out.reshape(B, 1, SPECW, N_MELS).astype(np.float32)
